# revision 28
# baseline (speedup 1.0000x reference)
"""Trainium2 Bass kernel for nn_CombinedLoss (CE + contrastive loss).

Data-parallel over the batch dim: 4 batches per core on 8 NeuronCores.
Each core returns partial (cls_sum, cls_cnt, con_sum, con_cnt); the host
reduces across cores and performs the final divisions.

v2: fp8e4+DoubleRow sim matmuls, raw-g transpose (g's norm folded into
the exp's per-row scale), chunked HWDGE DMA, elementwise work spread
across DVE/ACT/Pool.
"""

import os
import sys

for _p in ("/opt/trn_rl_repo", "/root/.axon_site/_ro/trn_rl_repo"):
    if os.path.isdir(_p) and _p not in sys.path:
        sys.path.insert(0, _p)

import math
from contextlib import ExitStack

import numpy as np

import concourse.bass as bass
import concourse.bacc as bacc
import concourse.tile as tile
from concourse import mybir

B, P, H = 32, 1024, 768
NCORES = 8
BPC = B // NCORES          # batches per core
MC = P // 128              # 128-token chunks per batch
KC = H // 128              # 128-dim contraction chunks
K3 = KC // 2               # 256-dim DoubleRow contraction chunks
TEMP = 0.07
F32 = mybir.dt.float32
BF16 = mybir.dt.bfloat16
F8 = mybir.dt.float8e4
DR = mybir.MatmulPerfMode.DoubleRow


def _emit(ctx, tc, out_d, g_d, e_d, lg_d, lab_d, eye_d):
    nc = tc.nc
    AL = mybir.AluOpType
    AF = mybir.ActivationFunctionType
    AX = mybir.AxisListType

    consts = ctx.enter_context(tc.tile_pool(name="consts", bufs=1))
    nat = ctx.enter_context(tc.tile_pool(name="nat", bufs=3))
    trans = ctx.enter_context(tc.tile_pool(name="trans", bufs=2))
    diagp = ctx.enter_context(tc.tile_pool(name="diagp", bufs=2))
    small = ctx.enter_context(tc.tile_pool(name="small", bufs=2))
    scrp = ctx.enter_context(tc.tile_pool(name="scrp", bufs=4))
    expp = ctx.enter_context(tc.tile_pool(name="expp", bufs=2))
    ps_sim = ctx.enter_context(tc.tile_pool(name="ps_sim", bufs=2, space="PSUM"))
    ps_tr = ctx.enter_context(tc.tile_pool(name="ps_tr", bufs=3, space="PSUM"))
    ps_sm = ctx.enter_context(tc.tile_pool(name="ps_sm", bufs=1, space="PSUM"))

    # small latency-sensitive DMAs first, on the sync (HWDGE) ring
    eye = consts.tile([128, 128], F32)
    nc.sync.dma_start(out=eye, in_=eye_d)
    eye_bf = consts.tile([128, 128], BF16)
    nc.gpsimd.dma_start(out=eye_bf, in_=eye_d)    # cast f32 -> bf16 (SWDGE)
    lgt = consts.tile([128, 2 * P * BPC // 128], F32)          # [128, 64]
    nc.sync.dma_start(
        out=lgt,
        in_=lg_d.rearrange("b p y -> (b p y)").rearrange("(q f) -> q f", q=128),
    )
    labfl = consts.tile([128, P * BPC // 128], F32)            # [128, 32]
    nc.sync.dma_start(
        out=labfl,
        in_=lab_d.rearrange("b p -> (b p)").rearrange("(q f) -> q f", q=128),
    )
    # all batches' labels in [m, (b q)] layout, one upfront DMA
    lab8x4 = consts.tile([8, BPC * 128], F32)
    nc.sync.dma_start(
        out=lab8x4.rearrange("m (b q) -> m b q", q=128),
        in_=lab_d.rearrange("b (m q) -> m b q", q=128),
    )
    ones_col = consts.tile([128, 1], F32)
    nc.vector.memset(ones_col, 1.0)
    ones_row = consts.tile([1, 128], F32)
    nc.vector.memset(ones_row, 1.0)

    c_lnT = consts.tile([128, 1], F32)                 # ln(1/TEMP) bias for ACT
    nc.vector.memset(c_lnT, float(math.log(1.0 / TEMP)))

    # column accumulators; final partition-sum happens once at the end
    acc4 = consts.tile([128, 4], F32)          # cls_sum | cls_cnt | con_sum | con_cnt
    con_sum_parts = consts.tile([128, BPC], F32)
    con_cnt_parts = consts.tile([128, BPC], F32)

    # ---------------- nat DMA, chunked on the gpsimd (SWDGE) ring -----------------
    nat_tiles = {}

    def emit_dma(b):
        # e entirely first: its arrival gates the critical chain
        # (sse -> inve -> de -> e-trans); g only feeds later stages
        g_nat = nat.tile([128, MC * H], BF16, tag="g_nat", name="g_nat")
        e_nat = nat.tile([128, MC * H], BF16, tag="e_nat", name="e_nat")
        esplit = 4 if b == 0 else 2
        for nt, dd, nsplit in ((e_nat, e_d, esplit), (g_nat, g_d, 2)):
            mm = MC // nsplit
            for hh in range(nsplit):
                nc.gpsimd.dma_start(
                    out=nt[:, hh * mm * H:(hh + 1) * mm * H]
                        .rearrange("q (m h) -> q m h", m=mm),
                    in_=dd[b][hh * mm * 128:(hh + 1) * mm * 128]
                        .rearrange("(m q) h -> q m h", q=128))
        nat_tiles[b] = (g_nat, e_nat)

    emit_dma(0)
    emit_dma(1)

    # ---------------- classification CE (tiny, fills the ramp) -----------------
    lg3 = lgt.rearrange("q (t y) -> q t y", y=2)
    x0 = lg3[:, :, 0:1].rearrange("q t y -> q (t y)")          # [128, 32] strided
    x1 = lg3[:, :, 1:2].rearrange("q t y -> q (t y)")

    nctok = P * BPC // 128                                     # 32
    e0 = consts.tile([128, nctok], F32)
    nc.scalar.activation(e0, x0, AF.Exp)
    e1 = consts.tile([128, nctok], F32)
    nc.scalar.activation(e1, x1, AF.Exp)
    se = consts.tile([128, nctok], F32)
    nc.vector.tensor_add(se, e0, e1)
    lae = consts.tile([128, nctok], F32)
    nc.scalar.activation(lae, se, AF.Ln)                   # logaddexp(x0, x1)
    validm = consts.tile([128, nctok], F32)
    nc.vector.tensor_scalar(validm, labfl, 0.0, None, AL.is_ge)
    tv = consts.tile([128, nctok], F32)
    nc.vector.tensor_mul(tv, labfl, validm)                # target as {0,1}
    d10 = consts.tile([128, nctok], F32)
    nc.vector.tensor_sub(d10, x1, x0)
    td = consts.tile([128, nctok], F32)
    nc.vector.tensor_mul(td, tv, d10)
    xt = consts.tile([128, nctok], F32)
    nc.vector.tensor_add(xt, x0, td)                       # x_target
    ce = consts.tile([128, nctok], F32)
    nc.vector.tensor_sub(ce, lae, xt)
    clsscr = consts.tile([128, nctok], F32)
    nc.vector.scalar_tensor_tensor(
        out=clsscr, in0=ce, scalar=1.0, in1=validm,
        op0=AL.mult, op1=AL.mult, accum_out=acc4[:, 0:1],
    )
    nc.vector.tensor_reduce(acc4[:, 1:2], validm, AX.X, AL.add)

    # ---------------- contrastive loss, software-pipelined -----------------
    # gt: raw g transposed (norm applied later via the exp's per-row scale)
    # et: e transposed, scaled per-token by negmask * inv_norm_e / TEMP
    # Both fp8e4 with DoubleRow layout [128, (kk=k3*2+ko)*1024 + tok].
    # ssg comes from diag(gt.T @ gt) on the PE (fp8-consistent norms).

    def emit_masks(b):
        st = {}
        # labels in column form [128, 8]: token 128*m + p at [p, m]
        ps_lab = ps_sm.tile([128, 8], F32, tag="sm", name="ps_lab")
        nc.tensor.transpose(ps_lab, lab8x4[:, b * 128:(b + 1) * 128],
                            eye[0:8, 0:8])
        lab_col = small.tile([128, MC], F32, tag="lab_col", name="lab_col")
        nc.vector.tensor_copy(lab_col, ps_lab)
        posm = small.tile([128, MC], F32, tag="posm", name="posm")
        nc.vector.tensor_scalar(posm, lab_col, 1.0, None, AL.is_equal)
        negm = small.tile([128, MC], F32, tag="negm", name="negm")
        nc.vector.tensor_scalar(negm, lab_col, 0.0, None, AL.is_equal)
        st.update(posm=posm, negm=negm)
        return st

    def emit_sse(b, st):
        e_nat = nat_tiles[b][1]
        sse = small.tile([128, MC], F32, tag="sse", name="sse")
        for m in range(MC):
            es = e_nat[:, m * H:(m + 1) * H]
            if m % 2 == 0:
                scr_e = scrp.tile([128, H], BF16, tag="scr_act", name="scr_e")
                nc.scalar.activation(out=scr_e, in_=es, func=AF.Square,
                                     accum_out=sse[:, m:m + 1])
            else:
                # split mult (2x mode) + scalar-accum (4x mode): faster than
                # one scalar_tensor_tensor (no DVE fast modes)
                sq_e = scrp.tile([128, H], BF16, tag="scr_dve", name="sq_e")
                nc.vector.tensor_mul(sq_e, es, es)
                scr_e = scrp.tile([128, H], BF16, tag="scr_dv2", name="scr_e")
                nc.vector.tensor_scalar(scr_e, sq_e, 1.0, 0.0, AL.mult,
                                        AL.add, accum_out=sse[:, m:m + 1])
        st.update(sse=sse)

    def emit_echain(b, st):
        sse, negm, posm = st["sse"], st["negm"], st["posm"]
        # inve_T = exp(-0.5 ln sse + ln(1/T)); inve_eff = inve_T * negm
        lne = small.tile([128, MC], F32, tag="lne", name="lne")
        inve_T = small.tile([128, MC], F32, tag="inve_T", name="inve_T")
        inve_eff = small.tile([128, MC], F32, tag="inve_eff", name="inve_eff")
        nc.scalar.activation(lne, sse, AF.Ln)
        nc.scalar.activation(inve_T, lne, AF.Exp, scale=-0.5, bias=c_lnT)
        nc.vector.tensor_mul(inve_eff, inve_T, negm)

        # per-token diagonal scale matrix for e's fused scaled transpose
        de = diagp.tile([128, MC * 128], BF16, tag="de", name="de")
        for m in range(MC):
            nc.vector.tensor_scalar(de[:, m * 128:(m + 1) * 128], eye_bf,
                                    inve_eff[:, m:m + 1], None, AL.mult)

        # batch_ok and the "zeroed columns contribute exp(0)=1" correction
        cnt2 = small.tile([128, 2], F32, tag="cnt2", name="cnt2")
        nc.vector.tensor_reduce(cnt2[:, 0:1], negm, AX.X, AL.add)
        nc.vector.tensor_reduce(cnt2[:, 1:2], posm, AX.X, AL.add)
        ps_cnt = ps_sm.tile([128, 8], F32, tag="sm", name="ps_cnt")
        nc.tensor.matmul(ps_cnt[0:1, 0:2], lhsT=ones_col, rhs=cnt2,
                         start=True, stop=True)
        cnt_sb = small.tile([1, 2], F32, tag="cnt_sb", name="cnt_sb")
        nc.vector.tensor_copy(cnt_sb, ps_cnt[0:1, 0:2])
        mn = small.tile([1, 2], F32, tag="mn", name="mn")
        nc.vector.tensor_scalar(mn, cnt_sb, 1.0, None, AL.min)
        okn = small.tile([1, 2], F32, tag="okn", name="okn")  # [ok, P - n_neg]
        nc.vector.tensor_mul(okn[:, 0:1], mn[:, 0:1], mn[:, 1:2])
        nc.vector.tensor_scalar(okn[:, 1:2], cnt_sb[:, 0:1], -1.0, float(P),
                                AL.mult, AL.add)
        ps_bc = ps_sm.tile([128, 8], F32, tag="sm", name="ps_bc")
        nc.tensor.matmul(ps_bc[:, 0:2], lhsT=ones_row, rhs=okn,
                         start=True, stop=True)
        bc_sb = small.tile([128, 2], F32, tag="bc_sb", name="bc_sb")
        nc.vector.tensor_copy(bc_sb, ps_bc[:, 0:2])
        st.update(inve_T=inve_T, bc_sb=bc_sb, de=de)

    def emit_etrans(b, st):
        e_nat = nat_tiles[b][1]
        de = st["de"]
        et = trans.tile([128, KC * P], F8, tag="et", name="et")
        cp = 0
        for half in range(2):       # h0 chunks first: matches DMA arrival
            for kk in range(KC):
                pt = ps_tr.tile([128, 512], F32, tag="pt", name="pt")
                for mi in range(4):
                    m = half * 4 + mi
                    nc.tensor.matmul(
                        pt[:, mi * 128:(mi + 1) * 128],
                        lhsT=e_nat[:, m * H + kk * 128: m * H + (kk + 1) * 128],
                        rhs=de[:, m * 128:(m + 1) * 128],
                        start=True, stop=True,
                    )
                dst = et[:, kk * P + half * 512: kk * P + half * 512 + 512]
                if cp % 2 == 0:
                    nc.scalar.copy(out=dst, in_=pt)
                else:
                    nc.vector.tensor_copy(dst, pt)
                cp += 1
        st.update(et=et)

    def emit_gtrans(b, st):
        # unscaled transpose of raw g into fp8 DoubleRow layout
        g_nat = nat_tiles[b][0]
        gt = trans.tile([128, KC * P], F8, tag="gt", name="gt")
        cp = 0
        for half in range(2):       # h0 chunks first: matches DMA arrival
            for kk in range(KC):
                pt = ps_tr.tile([128, 512], F32, tag="pt", name="pt")
                for mi in range(4):
                    m = half * 4 + mi
                    nc.tensor.matmul(
                        pt[:, mi * 128:(mi + 1) * 128],
                        lhsT=g_nat[:, m * H + kk * 128: m * H + (kk + 1) * 128],
                        rhs=eye_bf,
                        start=True, stop=True,
                    )
                dst = gt[:, kk * P + half * 512: kk * P + half * 512 + 512]
                if cp % 2 == 0:
                    nc.scalar.copy(out=dst, in_=pt)
                else:
                    nc.vector.tensor_copy(dst, pt)
                cp += 1
        st.update(gt3=gt.rearrange("q (k3 ko t) -> q k3 ko t", ko=2, t=P))

    def emit_ssgpe(b, st):
        # ssg[t] = sum_h gt[h,t]^2 = diag(gt.T @ gt), DR matmuls + eye-masked
        # extraction; invg computed per half so exps can start early
        gt3 = st["gt3"]
        ssg = small.tile([128, MC], F32, tag="ssg", name="ssg")
        invg = small.tile([128, MC], F32, tag="invg", name="invg")
        lng = small.tile([128, MC], F32, tag="lng", name="lng")
        for m in range(MC):
            psd = ps_tr.tile([128, 512], F32, tag="pt", name="psd")[:, 0:128]
            for k3 in range(K3):
                sl = gt3[:, k3, :, m * 128:(m + 1) * 128]
                nc.tensor.matmul(psd, lhsT=sl, rhs=sl,
                                 start=(k3 == 0), stop=(k3 == K3 - 1),
                                 perf_mode=DR)
            scr_d = scrp.tile([128, 128], BF16, tag="scr_ex", name="scr_d")
            nc.vector.scalar_tensor_tensor(
                out=scr_d, in0=psd, scalar=1.0, in1=eye,
                op0=AL.mult, op1=AL.mult, accum_out=ssg[:, m:m + 1],
            )
            if m % 4 == 3:
                hs = slice(m - 3, m + 1)
                nc.scalar.activation(lng[:, hs], ssg[:, hs], AF.Ln)
                nc.scalar.activation(invg[:, hs], lng[:, hs], AF.Exp,
                                     scale=-0.5)
        st.update(ssg=ssg, invg=invg)

    def emit_praw(b, st):
        # deferred into the NEXT iteration: praw would otherwise HOL-block
        # the next batch's de/e-chain on the DVE queue
        g_nat, e_nat = nat_tiles[b]
        praw = small.tile([128, MC], F32, tag="praw", name="praw")
        for m in range(MC):
            gs = g_nat[:, m * H:(m + 1) * H]
            es = e_nat[:, m * H:(m + 1) * H]
            sq_p = scrp.tile([128, H], BF16, tag="scr_pr", name="sq_p")
            nc.vector.tensor_mul(sq_p, gs, es)
            scr_p = scrp.tile([128, H], BF16, tag="scr_pr2", name="scr_p")
            nc.vector.tensor_scalar(scr_p, sq_p, 1.0, 0.0, AL.mult,
                                    AL.add, accum_out=praw[:, m:m + 1])
        pos = small.tile([128, MC], F32, tag="pos", name="pos")
        nc.vector.tensor_mul(pos, praw, st["invg"])
        nc.vector.tensor_mul(pos, pos, st["inve_T"])
        st.update(pos=pos)

    def emit_sims(b, st):
        gt3, et, invg = st["gt3"], st["et"], st["invg"]
        et3 = et.rearrange("q (k3 ko t) -> q k3 ko t", ko=2, t=P)
        s_col = small.tile([128, MC], F32, tag="s_col", name="s_col")
        for m in range(MC):
            ps = ps_sim.tile([128, P], F32, tag="ps", name="ps")
            for k3 in range(K3):
                for half in range(2):
                    nc.tensor.matmul(
                        ps[:, half * 512:(half + 1) * 512],
                        lhsT=gt3[:, k3, :, m * 128:(m + 1) * 128],
                        rhs=et3[:, k3, :, half * 512: half * 512 + 512],
                        start=(k3 == 0), stop=(k3 == K3 - 1),
                        perf_mode=DR,
                    )
            esc = expp.tile([128, P], BF16, tag="esc", name="esc")
            nc.scalar.activation(out=esc, in_=ps, func=AF.Exp,
                                 scale=invg[:, m:m + 1],
                                 accum_out=s_col[:, m:m + 1])
        st.update(s_col=s_col)

    def emit_tail(b, st):
        # row_loss = ln(1 + s * exp(-pos)), masked by pos & batch_ok
        bc_sb, pos, posm, s_col = st["bc_sb"], st["pos"], st["posm"], st["s_col"]
        s_adj = small.tile([128, MC], F32, tag="s_adj", name="s_adj")
        nc.vector.tensor_scalar(s_adj, s_col, bc_sb[:, 1:2], None, AL.subtract)
        tn = small.tile([128, MC], F32, tag="tn", name="tn")
        nc.scalar.activation(tn, pos, AF.Exp, scale=-1.0)
        u = small.tile([128, MC], F32, tag="u", name="u")
        nc.vector.tensor_mul(u, s_adj, tn)
        v = small.tile([128, MC], F32, tag="v", name="v")
        nc.scalar.activation(v, u, AF.Ln, bias=1.0)
        meff = small.tile([128, MC], F32, tag="meff", name="meff")
        nc.vector.tensor_scalar(meff, posm, bc_sb[:, 0:1], None, AL.mult)
        scr8 = small.tile([128, MC], F32, tag="scr8", name="scr8")
        nc.vector.scalar_tensor_tensor(
            out=scr8, in0=v, scalar=1.0, in1=meff,
            op0=AL.mult, op1=AL.mult, accum_out=con_sum_parts[:, b:b + 1],
        )
        nc.vector.tensor_reduce(con_cnt_parts[:, b:b + 1], meff, AX.X, AL.add)

    prev = None
    for b in range(BPC):
        st = emit_masks(b)
        emit_sse(b, st)
        emit_echain(b, st)
        emit_etrans(b, st)
        if prev is not None:
            emit_praw(b - 1, prev)
        emit_gtrans(b, st)
        emit_ssgpe(b, st)
        if b + 2 < BPC:
            emit_dma(b + 2)
        if prev is not None:
            emit_tail(b - 1, prev)
        emit_sims(b, st)
        prev = st
    emit_praw(BPC - 1, prev)
    emit_tail(BPC - 1, prev)

    # ---------------- final partition reduction -----------------
    nc.vector.tensor_reduce(acc4[:, 2:3], con_sum_parts, AX.X, AL.add)
    nc.vector.tensor_reduce(acc4[:, 3:4], con_cnt_parts, AX.X, AL.add)
    ps_fin = ps_sm.tile([128, 8], F32, tag="sm")
    nc.tensor.matmul(ps_fin[0:1, 0:4], lhsT=ones_col, rhs=acc4,
                     start=True, stop=True)
    outsb = consts.tile([1, 4], F32)
    nc.vector.tensor_copy(outsb, ps_fin[0:1, 0:4])
    nc.sync.dma_start(out=out_d, in_=outsb)


def build_nc():
    nc = bacc.Bacc("TRN2", target_bir_lowering=False, debug=False)
    g_d = nc.dram_tensor("g", [BPC, P, H], F32, kind="ExternalInput").ap()
    e_d = nc.dram_tensor("e", [BPC, P, H], F32, kind="ExternalInput").ap()
    lg_d = nc.dram_tensor("lg", [BPC, P, 2], F32, kind="ExternalInput").ap()
    lab_d = nc.dram_tensor("lab", [BPC, P], F32, kind="ExternalInput").ap()
    eye_d = nc.dram_tensor("eye", [128, 128], F32, kind="ExternalInput").ap()
    out_d = nc.dram_tensor("out", [1, 4], F32, kind="ExternalOutput").ap()
    with tile.TileContext(nc) as tc:
        with ExitStack() as ctx:
            _emit(ctx, tc, out_d, g_d, e_d, lg_d, lab_d, eye_d)
    nc.compile()
    return nc


_NC_CACHE = {}


def _setup_pruned_act_tables():
    """Point walrus at an act-table dir containing only the one function set
    we use (exp/ln/square/copy), so it never ping-pongs ACT_TABLE_LOADs."""
    if os.environ.get("BASS_ACT_ROOT_JSON_PATH"):
        return
    try:
        import json
        import tempfile
        from neuronxcc.driver.Job import Job
        from neuronxcc.driver.jobs.support.FindActInfo import findActInfoFile
        src = findActInfoFile(Job.getPackageDir(), "gen3")
        src_dir = os.path.dirname(src)
        dst = os.path.join(tempfile.gettempdir(), "act_pruned_nle")
        os.makedirs(dst, exist_ok=True)
        for f in os.listdir(src_dir):
            d = os.path.join(dst, f)
            if not os.path.exists(d):
                os.symlink(os.path.join(src_dir, f), d)
        info = json.load(open(src))
        keep = [x for x in info["act_func_sets"]
                if x["name"] == "natural_log_exp_and_others"]
        if not keep:
            return
        info["act_func_sets"] = keep
        pruned = os.path.join(dst, "act_info.json")
        if os.path.islink(pruned) or os.path.exists(pruned):
            os.remove(pruned)
        json.dump(info, open(pruned, "w"))
        os.environ["BASS_ACT_ROOT_JSON_PATH"] = pruned

        # Bacc pre-places the table loads with set ids indexing the SAME
        # json walrus sees — patch its table source to the pruned file.
        import concourse.hw_specs as hw_specs
        if not getattr(hw_specs, "_act_tables_pruned", False):
            def _pruned_tables(module_arch, _p=pruned, _mb=mybir):
                with open(_p) as af:
                    ai = json.load(af)
                return {
                    ent["name"]: {
                        _mb.ActivationFunctionType.from_pwp(a)
                        for a in ent["act"].keys()
                    }
                    for ent in ai["act_func_sets"]
                }
            hw_specs.get_activation_tables = _pruned_tables
            bacc.get_activation_tables = _pruned_tables
            hw_specs._act_tables_pruned = True
    except Exception:
        os.environ.pop("BASS_ACT_ROOT_JSON_PATH", None)  # fall back to default


def _get_nc():
    if "nc" not in _NC_CACHE:
        _setup_pruned_act_tables()
        _NC_CACHE["nc"] = build_nc()
    return _NC_CACHE["nc"]


def make_in_maps(logits, labels, greek_embeds, english_embeds):
    logits = np.ascontiguousarray(np.asarray(logits), dtype=np.float32)
    labf = np.ascontiguousarray(np.asarray(labels)).astype(np.float32)
    g = np.ascontiguousarray(np.asarray(greek_embeds), dtype=np.float32)
    e = np.ascontiguousarray(np.asarray(english_embeds), dtype=np.float32)
    eye = np.eye(128, dtype=np.float32)
    in_maps = []
    for c in range(NCORES):
        sl = slice(c * BPC, (c + 1) * BPC)
        in_maps.append({
            "g": np.ascontiguousarray(g[sl]),
            "e": np.ascontiguousarray(e[sl]),
            "lg": np.ascontiguousarray(logits[sl]),
            "lab": np.ascontiguousarray(labf[sl]),
            "eye": eye,
        })
    return in_maps


def combine_outputs(results):
    parts = np.stack([np.asarray(r["out"]).reshape(4) for r in results]).astype(np.float64)
    cls_sum, cls_cnt, con_sum, con_cnt = parts.sum(axis=0)
    cls = cls_sum / max(cls_cnt, 1.0)
    con = 0.0 if con_cnt == 0 else con_sum / max(con_cnt, 1.0)
    return np.float32(1.0 * cls + 0.5 * con)


def kernel(logits, labels, greek_embeds, english_embeds):
    from concourse import bass_utils

    nc = _get_nc()
    in_maps = make_in_maps(logits, labels, greek_embeds, english_embeds)
    res = bass_utils.run_bass_kernel_spmd(nc, in_maps, core_ids=list(range(NCORES)))
    return combine_outputs(res.results)


# revision 30
# speedup vs baseline: 1.0539x; 1.0539x over previous
"""Trainium2 Bass kernel for nn_CombinedLoss (CE + contrastive loss).

Data-parallel over the batch dim: 4 batches per core on 8 NeuronCores.
Each core returns partial (cls_sum, cls_cnt, con_sum, con_cnt); the host
reduces across cores and performs the final divisions.

v2: fp8e4+DoubleRow sim matmuls, raw-g transpose (g's norm folded into
the exp's per-row scale), chunked HWDGE DMA, elementwise work spread
across DVE/ACT/Pool.
"""

import os
import sys

for _p in ("/opt/trn_rl_repo", "/root/.axon_site/_ro/trn_rl_repo"):
    if os.path.isdir(_p) and _p not in sys.path:
        sys.path.insert(0, _p)

import math
from contextlib import ExitStack

import numpy as np

import concourse.bass as bass
import concourse.bacc as bacc
import concourse.tile as tile
from concourse import mybir

B, P, H = 32, 1024, 768
NCORES = 8
BPC = B // NCORES          # batches per core
MC = P // 128              # 128-token chunks per batch
KC = H // 128              # 128-dim contraction chunks
K3 = KC // 2               # 256-dim DoubleRow contraction chunks
TEMP = 0.07
F32 = mybir.dt.float32
BF16 = mybir.dt.bfloat16
F8 = mybir.dt.float8e4
DR = mybir.MatmulPerfMode.DoubleRow


def _emit(ctx, tc, out_d, g_d, e_d, lg_d, lab_d, eye_d):
    nc = tc.nc
    AL = mybir.AluOpType
    AF = mybir.ActivationFunctionType
    AX = mybir.AxisListType

    consts = ctx.enter_context(tc.tile_pool(name="consts", bufs=1))
    nat = ctx.enter_context(tc.tile_pool(name="nat", bufs=3))
    trans = ctx.enter_context(tc.tile_pool(name="trans", bufs=2))
    diagp = ctx.enter_context(tc.tile_pool(name="diagp", bufs=2))
    small = ctx.enter_context(tc.tile_pool(name="small", bufs=2))
    scrp = ctx.enter_context(tc.tile_pool(name="scrp", bufs=4))
    expp = ctx.enter_context(tc.tile_pool(name="expp", bufs=2))
    ps_sim = ctx.enter_context(tc.tile_pool(name="ps_sim", bufs=2, space="PSUM"))
    ps_tr = ctx.enter_context(tc.tile_pool(name="ps_tr", bufs=3, space="PSUM"))
    ps_sm = ctx.enter_context(tc.tile_pool(name="ps_sm", bufs=1, space="PSUM"))

    # small latency-sensitive DMAs first, on the sync (HWDGE) ring
    eye = consts.tile([128, 128], F32)
    nc.sync.dma_start(out=eye, in_=eye_d)
    eye_bf = consts.tile([128, 128], BF16)
    nc.gpsimd.dma_start(out=eye_bf, in_=eye_d)    # cast f32 -> bf16 (SWDGE)
    lgt = consts.tile([128, 2 * P * BPC // 128], F32)          # [128, 64]
    nc.sync.dma_start(
        out=lgt,
        in_=lg_d.rearrange("b p y -> (b p y)").rearrange("(q f) -> q f", q=128),
    )
    labfl = consts.tile([128, P * BPC // 128], F32)            # [128, 32]
    nc.sync.dma_start(
        out=labfl,
        in_=lab_d.rearrange("b p -> (b p)").rearrange("(q f) -> q f", q=128),
    )
    # all batches' labels in [m, (b q)] layout, one upfront DMA
    lab8x4 = consts.tile([8, BPC * 128], F32)
    nc.sync.dma_start(
        out=lab8x4.rearrange("m (b q) -> m b q", q=128),
        in_=lab_d.rearrange("b (m q) -> m b q", q=128),
    )
    ones_col = consts.tile([128, 1], F32)
    nc.vector.memset(ones_col, 1.0)
    ones_row = consts.tile([1, 128], F32)
    nc.vector.memset(ones_row, 1.0)

    c_lnT = consts.tile([128, 1], F32)                 # ln(1/TEMP) bias for ACT
    nc.vector.memset(c_lnT, float(math.log(1.0 / TEMP)))

    # column accumulators; final partition-sum happens once at the end
    acc4 = consts.tile([128, 4], F32)          # cls_sum | cls_cnt | con_sum | con_cnt
    con_sum_parts = consts.tile([128, BPC], F32)
    con_cnt_parts = consts.tile([128, BPC], F32)

    # ---------------- nat DMA, chunked on the gpsimd (SWDGE) ring -----------------
    nat_tiles = {}

    def emit_dma(b):
        # e entirely first: its arrival gates the critical chain
        # (sse -> inve -> de -> e-trans); g only feeds later stages
        g_nat = nat.tile([128, MC * H], BF16, tag="g_nat", name="g_nat")
        e_nat = nat.tile([128, MC * H], BF16, tag="e_nat", name="e_nat")
        esplit = 4 if b == 0 else 2
        for nt, dd, nsplit in ((e_nat, e_d, esplit), (g_nat, g_d, 2)):
            mm = MC // nsplit
            for hh in range(nsplit):
                nc.gpsimd.dma_start(
                    out=nt[:, hh * mm * H:(hh + 1) * mm * H]
                        .rearrange("q (m h) -> q m h", m=mm),
                    in_=dd[b][hh * mm * 128:(hh + 1) * mm * 128]
                        .rearrange("(m q) h -> q m h", q=128))
        nat_tiles[b] = (g_nat, e_nat)

    emit_dma(0)
    emit_dma(1)

    # ---------------- classification CE (tiny, fills the ramp) -----------------
    lg3 = lgt.rearrange("q (t y) -> q t y", y=2)
    x0 = lg3[:, :, 0:1].rearrange("q t y -> q (t y)")          # [128, 32] strided
    x1 = lg3[:, :, 1:2].rearrange("q t y -> q (t y)")

    nctok = P * BPC // 128                                     # 32
    e0 = consts.tile([128, nctok], F32)
    nc.scalar.activation(e0, x0, AF.Exp)
    e1 = consts.tile([128, nctok], F32)
    nc.scalar.activation(e1, x1, AF.Exp)
    se = consts.tile([128, nctok], F32)
    nc.vector.tensor_add(se, e0, e1)
    lae = consts.tile([128, nctok], F32)
    nc.scalar.activation(lae, se, AF.Ln)                   # logaddexp(x0, x1)
    validm = consts.tile([128, nctok], F32)
    nc.vector.tensor_scalar(validm, labfl, 0.0, None, AL.is_ge)
    tv = consts.tile([128, nctok], F32)
    nc.vector.tensor_mul(tv, labfl, validm)                # target as {0,1}
    d10 = consts.tile([128, nctok], F32)
    nc.vector.tensor_sub(d10, x1, x0)
    td = consts.tile([128, nctok], F32)
    nc.vector.tensor_mul(td, tv, d10)
    xt = consts.tile([128, nctok], F32)
    nc.vector.tensor_add(xt, x0, td)                       # x_target
    ce = consts.tile([128, nctok], F32)
    nc.vector.tensor_sub(ce, lae, xt)
    clsscr = consts.tile([128, nctok], F32)
    nc.vector.scalar_tensor_tensor(
        out=clsscr, in0=ce, scalar=1.0, in1=validm,
        op0=AL.mult, op1=AL.mult, accum_out=acc4[:, 0:1],
    )
    nc.vector.tensor_reduce(acc4[:, 1:2], validm, AX.X, AL.add)

    # ---------------- contrastive loss, software-pipelined -----------------
    # gt: raw g transposed (norm applied later via the exp's per-row scale)
    # et: e transposed, scaled per-token by negmask * inv_norm_e / TEMP
    # Both fp8e4 with DoubleRow layout [128, (kk=k3*2+ko)*1024 + tok].
    # ssg comes from diag(gt.T @ gt) on the PE (fp8-consistent norms).

    def emit_masks(b):
        st = {}
        # labels in column form [128, 8]: token 128*m + p at [p, m]
        ps_lab = ps_sm.tile([128, 8], F32, tag="sm", name="ps_lab")
        nc.tensor.transpose(ps_lab, lab8x4[:, b * 128:(b + 1) * 128],
                            eye[0:8, 0:8])
        lab_col = small.tile([128, MC], F32, tag="lab_col", name="lab_col")
        nc.vector.tensor_copy(lab_col, ps_lab)
        posm = small.tile([128, MC], F32, tag="posm", name="posm")
        nc.vector.tensor_scalar(posm, lab_col, 1.0, None, AL.is_equal)
        negm = small.tile([128, MC], F32, tag="negm", name="negm")
        nc.vector.tensor_scalar(negm, lab_col, 0.0, None, AL.is_equal)
        st.update(posm=posm, negm=negm)
        return st

    def emit_sse(b, st):
        e_nat = nat_tiles[b][1]
        sse = small.tile([128, MC], F32, tag="sse", name="sse")
        for m in range(MC):
            es = e_nat[:, m * H:(m + 1) * H]
            if m % 2 == 0:
                scr_e = scrp.tile([128, H], BF16, tag="scr_act", name="scr_e")
                nc.scalar.activation(out=scr_e, in_=es, func=AF.Square,
                                     accum_out=sse[:, m:m + 1])
            else:
                scr_e = scrp.tile([128, H], BF16, tag="scr_dve", name="scr_e")
                nc.vector.scalar_tensor_tensor(
                    out=scr_e, in0=es, scalar=1.0, in1=es,
                    op0=AL.mult, op1=AL.mult, accum_out=sse[:, m:m + 1],
                )
        st.update(sse=sse)

    def emit_echain(b, st):
        sse, negm, posm = st["sse"], st["negm"], st["posm"]
        # inve_T = exp(-0.5 ln sse + ln(1/T)); inve_eff = inve_T * negm
        lne = small.tile([128, MC], F32, tag="lne", name="lne")
        inve_T = small.tile([128, MC], F32, tag="inve_T", name="inve_T")
        inve_eff = small.tile([128, MC], F32, tag="inve_eff", name="inve_eff")
        nc.scalar.activation(lne, sse, AF.Ln)
        nc.scalar.activation(inve_T, lne, AF.Exp, scale=-0.5, bias=c_lnT)
        nc.vector.tensor_mul(inve_eff, inve_T, negm)

        # per-token diagonal scale matrix for e's fused scaled transpose
        de = diagp.tile([128, MC * 128], BF16, tag="de", name="de")
        for m in range(MC):
            nc.vector.tensor_scalar(de[:, m * 128:(m + 1) * 128], eye_bf,
                                    inve_eff[:, m:m + 1], None, AL.mult)

        # batch_ok and the "zeroed columns contribute exp(0)=1" correction
        cnt2 = small.tile([128, 2], F32, tag="cnt2", name="cnt2")
        nc.vector.tensor_reduce(cnt2[:, 0:1], negm, AX.X, AL.add)
        nc.vector.tensor_reduce(cnt2[:, 1:2], posm, AX.X, AL.add)
        ps_cnt = ps_sm.tile([128, 8], F32, tag="sm", name="ps_cnt")
        nc.tensor.matmul(ps_cnt[0:1, 0:2], lhsT=ones_col, rhs=cnt2,
                         start=True, stop=True)
        cnt_sb = small.tile([1, 2], F32, tag="cnt_sb", name="cnt_sb")
        nc.vector.tensor_copy(cnt_sb, ps_cnt[0:1, 0:2])
        mn = small.tile([1, 2], F32, tag="mn", name="mn")
        nc.vector.tensor_scalar(mn, cnt_sb, 1.0, None, AL.min)
        okn = small.tile([1, 2], F32, tag="okn", name="okn")  # [ok, P - n_neg]
        nc.vector.tensor_mul(okn[:, 0:1], mn[:, 0:1], mn[:, 1:2])
        nc.vector.tensor_scalar(okn[:, 1:2], cnt_sb[:, 0:1], -1.0, float(P),
                                AL.mult, AL.add)
        ps_bc = ps_sm.tile([128, 8], F32, tag="sm", name="ps_bc")
        nc.tensor.matmul(ps_bc[:, 0:2], lhsT=ones_row, rhs=okn,
                         start=True, stop=True)
        bc_sb = small.tile([128, 2], F32, tag="bc_sb", name="bc_sb")
        nc.vector.tensor_copy(bc_sb, ps_bc[:, 0:2])
        st.update(inve_T=inve_T, bc_sb=bc_sb, de=de)

    def emit_etrans(b, st):
        e_nat = nat_tiles[b][1]
        de = st["de"]
        et = trans.tile([128, KC * P], F8, tag="et", name="et")
        cp = 0
        for half in range(2):       # h0 chunks first: matches DMA arrival
            for kk in range(KC):
                pt = ps_tr.tile([128, 512], F32, tag="pt", name="pt")
                for mi in range(4):
                    m = half * 4 + mi
                    nc.tensor.matmul(
                        pt[:, mi * 128:(mi + 1) * 128],
                        lhsT=e_nat[:, m * H + kk * 128: m * H + (kk + 1) * 128],
                        rhs=de[:, m * 128:(m + 1) * 128],
                        start=True, stop=True,
                    )
                dst = et[:, kk * P + half * 512: kk * P + half * 512 + 512]
                if cp % 2 == 0:
                    nc.scalar.copy(out=dst, in_=pt)
                else:
                    nc.vector.tensor_copy(dst, pt)
                cp += 1
        st.update(et=et)

    def emit_gtrans(b, st):
        # unscaled transpose of raw g into fp8 DoubleRow layout
        g_nat = nat_tiles[b][0]
        gt = trans.tile([128, KC * P], F8, tag="gt", name="gt")
        cp = 0
        for half in range(2):       # h0 chunks first: matches DMA arrival
            for kk in range(KC):
                pt = ps_tr.tile([128, 512], F32, tag="pt", name="pt")
                for mi in range(4):
                    m = half * 4 + mi
                    nc.tensor.matmul(
                        pt[:, mi * 128:(mi + 1) * 128],
                        lhsT=g_nat[:, m * H + kk * 128: m * H + (kk + 1) * 128],
                        rhs=eye_bf,
                        start=True, stop=True,
                    )
                dst = gt[:, kk * P + half * 512: kk * P + half * 512 + 512]
                if cp % 2 == 0:
                    nc.scalar.copy(out=dst, in_=pt)
                else:
                    nc.vector.tensor_copy(dst, pt)
                cp += 1
        st.update(gt3=gt.rearrange("q (k3 ko t) -> q k3 ko t", ko=2, t=P))

    def emit_ssgpe(b, st):
        # ssg[t] = sum_h gt[h,t]^2 = diag(gt.T @ gt), DR matmuls + eye-masked
        # extraction; invg computed per half so exps can start early
        gt3 = st["gt3"]
        ssg = small.tile([128, MC], F32, tag="ssg", name="ssg")
        invg = small.tile([128, MC], F32, tag="invg", name="invg")
        lng = small.tile([128, MC], F32, tag="lng", name="lng")
        for m in range(MC):
            psd = ps_tr.tile([128, 512], F32, tag="pt", name="psd")[:, 0:128]
            for k3 in range(K3):
                sl = gt3[:, k3, :, m * 128:(m + 1) * 128]
                nc.tensor.matmul(psd, lhsT=sl, rhs=sl,
                                 start=(k3 == 0), stop=(k3 == K3 - 1),
                                 perf_mode=DR)
            scr_d = scrp.tile([128, 128], BF16, tag="scr_ex", name="scr_d")
            nc.vector.scalar_tensor_tensor(
                out=scr_d, in0=psd, scalar=1.0, in1=eye,
                op0=AL.mult, op1=AL.mult, accum_out=ssg[:, m:m + 1],
            )
            if m % 4 == 3:
                hs = slice(m - 3, m + 1)
                nc.scalar.activation(lng[:, hs], ssg[:, hs], AF.Ln)
                nc.scalar.activation(invg[:, hs], lng[:, hs], AF.Exp,
                                     scale=-0.5)
        st.update(ssg=ssg, invg=invg)

    def emit_praw(b, st):
        # deferred into the NEXT iteration: praw would otherwise HOL-block
        # the next batch's de/e-chain on the DVE queue
        g_nat, e_nat = nat_tiles[b]
        praw = small.tile([128, MC], F32, tag="praw", name="praw")
        for m in range(MC):
            gs = g_nat[:, m * H:(m + 1) * H]
            es = e_nat[:, m * H:(m + 1) * H]
            scr_p = scrp.tile([128, H], BF16, tag="scr_pr", name="scr_p")
            nc.vector.scalar_tensor_tensor(
                out=scr_p, in0=gs, scalar=1.0, in1=es,
                op0=AL.mult, op1=AL.mult, accum_out=praw[:, m:m + 1],
            )
        pos = small.tile([128, MC], F32, tag="pos", name="pos")
        nc.vector.tensor_mul(pos, praw, st["invg"])
        nc.vector.tensor_mul(pos, pos, st["inve_T"])
        st.update(pos=pos)

    def emit_sims(b, st):
        gt3, et, invg = st["gt3"], st["et"], st["invg"]
        et3 = et.rearrange("q (k3 ko t) -> q k3 ko t", ko=2, t=P)
        s_col = small.tile([128, MC], F32, tag="s_col", name="s_col")
        for m in range(MC):
            ps = ps_sim.tile([128, P], F32, tag="ps", name="ps")
            for k3 in range(K3):
                for half in range(2):
                    nc.tensor.matmul(
                        ps[:, half * 512:(half + 1) * 512],
                        lhsT=gt3[:, k3, :, m * 128:(m + 1) * 128],
                        rhs=et3[:, k3, :, half * 512: half * 512 + 512],
                        start=(k3 == 0), stop=(k3 == K3 - 1),
                        perf_mode=DR,
                    )
            esc = expp.tile([128, P], BF16, tag="esc", name="esc")
            nc.scalar.activation(out=esc, in_=ps, func=AF.Exp,
                                 scale=invg[:, m:m + 1],
                                 accum_out=s_col[:, m:m + 1])
        st.update(s_col=s_col)

    def emit_tail(b, st):
        # row_loss = ln(1 + s * exp(-pos)), masked by pos & batch_ok
        bc_sb, pos, posm, s_col = st["bc_sb"], st["pos"], st["posm"], st["s_col"]
        s_adj = small.tile([128, MC], F32, tag="s_adj", name="s_adj")
        nc.vector.tensor_scalar(s_adj, s_col, bc_sb[:, 1:2], None, AL.subtract)
        tn = small.tile([128, MC], F32, tag="tn", name="tn")
        nc.scalar.activation(tn, pos, AF.Exp, scale=-1.0)
        u = small.tile([128, MC], F32, tag="u", name="u")
        nc.vector.tensor_mul(u, s_adj, tn)
        v = small.tile([128, MC], F32, tag="v", name="v")
        nc.scalar.activation(v, u, AF.Ln, bias=1.0)
        meff = small.tile([128, MC], F32, tag="meff", name="meff")
        nc.vector.tensor_scalar(meff, posm, bc_sb[:, 0:1], None, AL.mult)
        scr8 = small.tile([128, MC], F32, tag="scr8", name="scr8")
        nc.vector.scalar_tensor_tensor(
            out=scr8, in0=v, scalar=1.0, in1=meff,
            op0=AL.mult, op1=AL.mult, accum_out=con_sum_parts[:, b:b + 1],
        )
        nc.vector.tensor_reduce(con_cnt_parts[:, b:b + 1], meff, AX.X, AL.add)

    prev = None
    for b in range(BPC):
        st = emit_masks(b)
        emit_sse(b, st)
        emit_echain(b, st)
        emit_etrans(b, st)
        if prev is not None:
            emit_praw(b - 1, prev)
        emit_gtrans(b, st)
        emit_ssgpe(b, st)
        if b + 2 < BPC:
            emit_dma(b + 2)
        if prev is not None:
            emit_tail(b - 1, prev)
        emit_sims(b, st)
        prev = st
    emit_praw(BPC - 1, prev)
    emit_tail(BPC - 1, prev)

    # ---------------- final partition reduction -----------------
    nc.vector.tensor_reduce(acc4[:, 2:3], con_sum_parts, AX.X, AL.add)
    nc.vector.tensor_reduce(acc4[:, 3:4], con_cnt_parts, AX.X, AL.add)
    ps_fin = ps_sm.tile([128, 8], F32, tag="sm")
    nc.tensor.matmul(ps_fin[0:1, 0:4], lhsT=ones_col, rhs=acc4,
                     start=True, stop=True)
    outsb = consts.tile([1, 4], F32)
    nc.vector.tensor_copy(outsb, ps_fin[0:1, 0:4])
    nc.sync.dma_start(out=out_d, in_=outsb)


def build_nc():
    nc = bacc.Bacc("TRN2", target_bir_lowering=False, debug=False)
    g_d = nc.dram_tensor("g", [BPC, P, H], F32, kind="ExternalInput").ap()
    e_d = nc.dram_tensor("e", [BPC, P, H], F32, kind="ExternalInput").ap()
    lg_d = nc.dram_tensor("lg", [BPC, P, 2], F32, kind="ExternalInput").ap()
    lab_d = nc.dram_tensor("lab", [BPC, P], F32, kind="ExternalInput").ap()
    eye_d = nc.dram_tensor("eye", [128, 128], F32, kind="ExternalInput").ap()
    out_d = nc.dram_tensor("out", [1, 4], F32, kind="ExternalOutput").ap()
    with tile.TileContext(nc) as tc:
        with ExitStack() as ctx:
            _emit(ctx, tc, out_d, g_d, e_d, lg_d, lab_d, eye_d)
    nc.compile()
    return nc


_NC_CACHE = {}


def _setup_pruned_act_tables():
    """Point walrus at an act-table dir containing only the one function set
    we use (exp/ln/square/copy), so it never ping-pongs ACT_TABLE_LOADs."""
    if os.environ.get("BASS_ACT_ROOT_JSON_PATH"):
        return
    try:
        import json
        import tempfile
        from neuronxcc.driver.Job import Job
        from neuronxcc.driver.jobs.support.FindActInfo import findActInfoFile
        src = findActInfoFile(Job.getPackageDir(), "gen3")
        src_dir = os.path.dirname(src)
        dst = os.path.join(tempfile.gettempdir(), "act_pruned_nle")
        os.makedirs(dst, exist_ok=True)
        for f in os.listdir(src_dir):
            d = os.path.join(dst, f)
            if not os.path.exists(d):
                os.symlink(os.path.join(src_dir, f), d)
        info = json.load(open(src))
        keep = [x for x in info["act_func_sets"]
                if x["name"] == "natural_log_exp_and_others"]
        if not keep:
            return
        info["act_func_sets"] = keep
        pruned = os.path.join(dst, "act_info.json")
        if os.path.islink(pruned) or os.path.exists(pruned):
            os.remove(pruned)
        json.dump(info, open(pruned, "w"))
        os.environ["BASS_ACT_ROOT_JSON_PATH"] = pruned

        # Bacc pre-places the table loads with set ids indexing the SAME
        # json walrus sees — patch its table source to the pruned file.
        import concourse.hw_specs as hw_specs
        if not getattr(hw_specs, "_act_tables_pruned", False):
            def _pruned_tables(module_arch, _p=pruned, _mb=mybir):
                with open(_p) as af:
                    ai = json.load(af)
                return {
                    ent["name"]: {
                        _mb.ActivationFunctionType.from_pwp(a)
                        for a in ent["act"].keys()
                    }
                    for ent in ai["act_func_sets"]
                }
            hw_specs.get_activation_tables = _pruned_tables
            bacc.get_activation_tables = _pruned_tables
            hw_specs._act_tables_pruned = True
    except Exception:
        os.environ.pop("BASS_ACT_ROOT_JSON_PATH", None)  # fall back to default


def _get_nc():
    if "nc" not in _NC_CACHE:
        _setup_pruned_act_tables()
        _NC_CACHE["nc"] = build_nc()
    return _NC_CACHE["nc"]


def make_in_maps(logits, labels, greek_embeds, english_embeds):
    logits = np.ascontiguousarray(np.asarray(logits), dtype=np.float32)
    labf = np.ascontiguousarray(np.asarray(labels)).astype(np.float32)
    g = np.ascontiguousarray(np.asarray(greek_embeds), dtype=np.float32)
    e = np.ascontiguousarray(np.asarray(english_embeds), dtype=np.float32)
    eye = np.eye(128, dtype=np.float32)
    in_maps = []
    for c in range(NCORES):
        sl = slice(c * BPC, (c + 1) * BPC)
        in_maps.append({
            "g": np.ascontiguousarray(g[sl]),
            "e": np.ascontiguousarray(e[sl]),
            "lg": np.ascontiguousarray(logits[sl]),
            "lab": np.ascontiguousarray(labf[sl]),
            "eye": eye,
        })
    return in_maps


def combine_outputs(results):
    parts = np.stack([np.asarray(r["out"]).reshape(4) for r in results]).astype(np.float64)
    cls_sum, cls_cnt, con_sum, con_cnt = parts.sum(axis=0)
    cls = cls_sum / max(cls_cnt, 1.0)
    con = 0.0 if con_cnt == 0 else con_sum / max(con_cnt, 1.0)
    return np.float32(1.0 * cls + 0.5 * con)


def kernel(logits, labels, greek_embeds, english_embeds):
    from concourse import bass_utils

    nc = _get_nc()
    in_maps = make_in_maps(logits, labels, greek_embeds, english_embeds)
    res = bass_utils.run_bass_kernel_spmd(nc, in_maps, core_ids=list(range(NCORES)))
    return combine_outputs(res.results)


# revision 31
# speedup vs baseline: 1.0724x; 1.0175x over previous
"""Trainium2 Bass kernel for nn_CombinedLoss (CE + contrastive loss).

Data-parallel over the batch dim: 4 batches per core on 8 NeuronCores.
Each core returns partial (cls_sum, cls_cnt, con_sum, con_cnt); the host
reduces across cores and performs the final divisions.

v2: fp8e4+DoubleRow sim matmuls, raw-g transpose (g's norm folded into
the exp's per-row scale), chunked HWDGE DMA, elementwise work spread
across DVE/ACT/Pool.
"""

import os
import sys

for _p in ("/opt/trn_rl_repo", "/root/.axon_site/_ro/trn_rl_repo"):
    if os.path.isdir(_p) and _p not in sys.path:
        sys.path.insert(0, _p)

import math
from contextlib import ExitStack

import numpy as np

import concourse.bass as bass
import concourse.bacc as bacc
import concourse.tile as tile
from concourse import mybir

B, P, H = 32, 1024, 768
NCORES = 8
BPC = B // NCORES          # batches per core
MC = P // 128              # 128-token chunks per batch
KC = H // 128              # 128-dim contraction chunks
K3 = KC // 2               # 256-dim DoubleRow contraction chunks
TEMP = 0.07
F32 = mybir.dt.float32
BF16 = mybir.dt.bfloat16
F8 = mybir.dt.float8e4
DR = mybir.MatmulPerfMode.DoubleRow


def _emit(ctx, tc, out_d, g_d, e_d, lg_d, lab_d, eye_d):
    nc = tc.nc
    AL = mybir.AluOpType
    AF = mybir.ActivationFunctionType
    AX = mybir.AxisListType

    consts = ctx.enter_context(tc.tile_pool(name="consts", bufs=1))
    nat = ctx.enter_context(tc.tile_pool(name="nat", bufs=3))
    trans = ctx.enter_context(tc.tile_pool(name="trans", bufs=2))
    diagp = ctx.enter_context(tc.tile_pool(name="diagp", bufs=2))
    small = ctx.enter_context(tc.tile_pool(name="small", bufs=2))
    scrp = ctx.enter_context(tc.tile_pool(name="scrp", bufs=4))
    expp = ctx.enter_context(tc.tile_pool(name="expp", bufs=2))
    ps_sim = ctx.enter_context(tc.tile_pool(name="ps_sim", bufs=2, space="PSUM"))
    ps_tr = ctx.enter_context(tc.tile_pool(name="ps_tr", bufs=3, space="PSUM"))
    ps_sm = ctx.enter_context(tc.tile_pool(name="ps_sm", bufs=1, space="PSUM"))

    # small latency-sensitive DMAs first, on the sync (HWDGE) ring
    eye = consts.tile([128, 128], F32)
    nc.sync.dma_start(out=eye, in_=eye_d)
    eye_bf = consts.tile([128, 128], BF16)
    nc.gpsimd.dma_start(out=eye_bf, in_=eye_d)    # cast f32 -> bf16 (SWDGE)
    lgt = consts.tile([128, 2 * P * BPC // 128], F32)          # [128, 64]
    nc.sync.dma_start(
        out=lgt,
        in_=lg_d.rearrange("b p y -> (b p y)").rearrange("(q f) -> q f", q=128),
    )
    labfl = consts.tile([128, P * BPC // 128], F32)            # [128, 32]
    nc.sync.dma_start(
        out=labfl,
        in_=lab_d.rearrange("b p -> (b p)").rearrange("(q f) -> q f", q=128),
    )
    # all batches' labels in [m, (b q)] layout, one upfront DMA
    lab8x4 = consts.tile([8, BPC * 128], F32)
    nc.sync.dma_start(
        out=lab8x4.rearrange("m (b q) -> m b q", q=128),
        in_=lab_d.rearrange("b (m q) -> m b q", q=128),
    )
    ones_col = consts.tile([128, 1], F32)
    nc.vector.memset(ones_col, 1.0)
    ones_row = consts.tile([1, 128], F32)
    nc.vector.memset(ones_row, 1.0)

    c_lnT = consts.tile([128, 1], F32)                 # ln(1/TEMP) bias for ACT
    nc.vector.memset(c_lnT, float(math.log(1.0 / TEMP)))

    # column accumulators; final partition-sum happens once at the end
    acc4 = consts.tile([128, 4], F32)          # cls_sum | cls_cnt | con_sum | con_cnt
    con_sum_parts = consts.tile([128, BPC], F32)
    con_cnt_parts = consts.tile([128, BPC], F32)

    # ---------------- nat DMA, chunked on the gpsimd (SWDGE) ring -----------------
    nat_tiles = {}

    def emit_dma(b):
        # e entirely first: its arrival gates the critical chain
        # (sse -> inve -> de -> e-trans); g only feeds later stages
        g_nat = nat.tile([128, MC * H], BF16, tag="g_nat", name="g_nat")
        e_nat = nat.tile([128, MC * H], BF16, tag="e_nat", name="e_nat")
        for nt, dd, nsplit in ((e_nat, e_d, 4), (g_nat, g_d, 2)):
            mm = MC // nsplit
            for hh in range(nsplit):
                nc.gpsimd.dma_start(
                    out=nt[:, hh * mm * H:(hh + 1) * mm * H]
                        .rearrange("q (m h) -> q m h", m=mm),
                    in_=dd[b][hh * mm * 128:(hh + 1) * mm * 128]
                        .rearrange("(m q) h -> q m h", q=128))
        nat_tiles[b] = (g_nat, e_nat)

    emit_dma(0)
    emit_dma(1)

    # ---------------- classification CE (tiny, fills the ramp) -----------------
    lg3 = lgt.rearrange("q (t y) -> q t y", y=2)
    x0 = lg3[:, :, 0:1].rearrange("q t y -> q (t y)")          # [128, 32] strided
    x1 = lg3[:, :, 1:2].rearrange("q t y -> q (t y)")

    nctok = P * BPC // 128                                     # 32
    e0 = consts.tile([128, nctok], F32)
    nc.scalar.activation(e0, x0, AF.Exp)
    e1 = consts.tile([128, nctok], F32)
    nc.scalar.activation(e1, x1, AF.Exp)
    se = consts.tile([128, nctok], F32)
    nc.vector.tensor_add(se, e0, e1)
    lae = consts.tile([128, nctok], F32)
    nc.scalar.activation(lae, se, AF.Ln)                   # logaddexp(x0, x1)
    validm = consts.tile([128, nctok], F32)
    nc.vector.tensor_scalar(validm, labfl, 0.0, None, AL.is_ge)
    tv = consts.tile([128, nctok], F32)
    nc.vector.tensor_mul(tv, labfl, validm)                # target as {0,1}
    d10 = consts.tile([128, nctok], F32)
    nc.vector.tensor_sub(d10, x1, x0)
    td = consts.tile([128, nctok], F32)
    nc.vector.tensor_mul(td, tv, d10)
    xt = consts.tile([128, nctok], F32)
    nc.vector.tensor_add(xt, x0, td)                       # x_target
    ce = consts.tile([128, nctok], F32)
    nc.vector.tensor_sub(ce, lae, xt)
    clsscr = consts.tile([128, nctok], F32)
    nc.vector.scalar_tensor_tensor(
        out=clsscr, in0=ce, scalar=1.0, in1=validm,
        op0=AL.mult, op1=AL.mult, accum_out=acc4[:, 0:1],
    )
    nc.vector.tensor_reduce(acc4[:, 1:2], validm, AX.X, AL.add)

    # ---------------- contrastive loss, software-pipelined -----------------
    # gt: raw g transposed (norm applied later via the exp's per-row scale)
    # et: e transposed, scaled per-token by negmask * inv_norm_e / TEMP
    # Both fp8e4 with DoubleRow layout [128, (kk=k3*2+ko)*1024 + tok].
    # ssg comes from diag(gt.T @ gt) on the PE (fp8-consistent norms).

    def emit_masks(b):
        st = {}
        # labels in column form [128, 8]: token 128*m + p at [p, m]
        ps_lab = ps_sm.tile([128, 8], F32, tag="sm", name="ps_lab")
        nc.tensor.transpose(ps_lab, lab8x4[:, b * 128:(b + 1) * 128],
                            eye[0:8, 0:8])
        lab_col = small.tile([128, MC], F32, tag="lab_col", name="lab_col")
        nc.vector.tensor_copy(lab_col, ps_lab)
        posm = small.tile([128, MC], F32, tag="posm", name="posm")
        nc.vector.tensor_scalar(posm, lab_col, 1.0, None, AL.is_equal)
        negm = small.tile([128, MC], F32, tag="negm", name="negm")
        nc.vector.tensor_scalar(negm, lab_col, 0.0, None, AL.is_equal)
        st.update(posm=posm, negm=negm)
        return st

    def emit_sse(b, st):
        e_nat = nat_tiles[b][1]
        sse = small.tile([128, MC], F32, tag="sse", name="sse")
        for m in range(MC):
            es = e_nat[:, m * H:(m + 1) * H]
            if m % 2 == 0:
                scr_e = scrp.tile([128, H], BF16, tag="scr_act", name="scr_e")
                nc.scalar.activation(out=scr_e, in_=es, func=AF.Square,
                                     accum_out=sse[:, m:m + 1])
            else:
                scr_e = scrp.tile([128, H], BF16, tag="scr_dve", name="scr_e")
                nc.vector.scalar_tensor_tensor(
                    out=scr_e, in0=es, scalar=1.0, in1=es,
                    op0=AL.mult, op1=AL.mult, accum_out=sse[:, m:m + 1],
                )
        st.update(sse=sse)

    def emit_echain(b, st):
        sse, negm, posm = st["sse"], st["negm"], st["posm"]
        # inve_T = exp(-0.5 ln sse + ln(1/T)); inve_eff = inve_T * negm
        lne = small.tile([128, MC], F32, tag="lne", name="lne")
        inve_T = small.tile([128, MC], F32, tag="inve_T", name="inve_T")
        inve_eff = small.tile([128, MC], F32, tag="inve_eff", name="inve_eff")
        nc.scalar.activation(lne, sse, AF.Ln)
        nc.scalar.activation(inve_T, lne, AF.Exp, scale=-0.5, bias=c_lnT)
        nc.vector.tensor_mul(inve_eff, inve_T, negm)

        # per-token diagonal scale matrix for e's fused scaled transpose
        de = diagp.tile([128, MC * 128], BF16, tag="de", name="de")
        for m in range(MC):
            nc.vector.tensor_scalar(de[:, m * 128:(m + 1) * 128], eye_bf,
                                    inve_eff[:, m:m + 1], None, AL.mult)

        # batch_ok and the "zeroed columns contribute exp(0)=1" correction
        cnt2 = small.tile([128, 2], F32, tag="cnt2", name="cnt2")
        nc.vector.tensor_reduce(cnt2[:, 0:1], negm, AX.X, AL.add)
        nc.vector.tensor_reduce(cnt2[:, 1:2], posm, AX.X, AL.add)
        ps_cnt = ps_sm.tile([128, 8], F32, tag="sm", name="ps_cnt")
        nc.tensor.matmul(ps_cnt[0:1, 0:2], lhsT=ones_col, rhs=cnt2,
                         start=True, stop=True)
        cnt_sb = small.tile([1, 2], F32, tag="cnt_sb", name="cnt_sb")
        nc.vector.tensor_copy(cnt_sb, ps_cnt[0:1, 0:2])
        mn = small.tile([1, 2], F32, tag="mn", name="mn")
        nc.vector.tensor_scalar(mn, cnt_sb, 1.0, None, AL.min)
        okn = small.tile([1, 2], F32, tag="okn", name="okn")  # [ok, P - n_neg]
        nc.vector.tensor_mul(okn[:, 0:1], mn[:, 0:1], mn[:, 1:2])
        nc.vector.tensor_scalar(okn[:, 1:2], cnt_sb[:, 0:1], -1.0, float(P),
                                AL.mult, AL.add)
        ps_bc = ps_sm.tile([128, 8], F32, tag="sm", name="ps_bc")
        nc.tensor.matmul(ps_bc[:, 0:2], lhsT=ones_row, rhs=okn,
                         start=True, stop=True)
        bc_sb = small.tile([128, 2], F32, tag="bc_sb", name="bc_sb")
        nc.vector.tensor_copy(bc_sb, ps_bc[:, 0:2])
        st.update(inve_T=inve_T, bc_sb=bc_sb, de=de)

    def emit_etrans(b, st):
        e_nat = nat_tiles[b][1]
        de = st["de"]
        et = trans.tile([128, KC * P], F8, tag="et", name="et")
        cp = 0
        for half in range(2):       # h0 chunks first: matches DMA arrival
            for kk in range(KC):
                pt = ps_tr.tile([128, 512], F32, tag="pt", name="pt")
                for mi in range(4):
                    m = half * 4 + mi
                    nc.tensor.matmul(
                        pt[:, mi * 128:(mi + 1) * 128],
                        lhsT=e_nat[:, m * H + kk * 128: m * H + (kk + 1) * 128],
                        rhs=de[:, m * 128:(m + 1) * 128],
                        start=True, stop=True,
                    )
                dst = et[:, kk * P + half * 512: kk * P + half * 512 + 512]
                if cp % 2 == 0:
                    nc.scalar.copy(out=dst, in_=pt)
                else:
                    nc.vector.tensor_copy(dst, pt)
                cp += 1
        st.update(et=et)

    def emit_gtrans(b, st):
        # unscaled transpose of raw g into fp8 DoubleRow layout
        g_nat = nat_tiles[b][0]
        gt = trans.tile([128, KC * P], F8, tag="gt", name="gt")
        cp = 0
        for half in range(2):       # h0 chunks first: matches DMA arrival
            for kk in range(KC):
                pt = ps_tr.tile([128, 512], F32, tag="pt", name="pt")
                for mi in range(4):
                    m = half * 4 + mi
                    nc.tensor.matmul(
                        pt[:, mi * 128:(mi + 1) * 128],
                        lhsT=g_nat[:, m * H + kk * 128: m * H + (kk + 1) * 128],
                        rhs=eye_bf,
                        start=True, stop=True,
                    )
                dst = gt[:, kk * P + half * 512: kk * P + half * 512 + 512]
                if cp % 2 == 0:
                    nc.scalar.copy(out=dst, in_=pt)
                else:
                    nc.vector.tensor_copy(dst, pt)
                cp += 1
        st.update(gt3=gt.rearrange("q (k3 ko t) -> q k3 ko t", ko=2, t=P))

    def emit_ssgpe(b, st):
        # ssg[t] = sum_h gt[h,t]^2 = diag(gt.T @ gt), DR matmuls + eye-masked
        # extraction; invg computed per half so exps can start early
        gt3 = st["gt3"]
        ssg = small.tile([128, MC], F32, tag="ssg", name="ssg")
        invg = small.tile([128, MC], F32, tag="invg", name="invg")
        lng = small.tile([128, MC], F32, tag="lng", name="lng")
        for m in range(MC):
            psd = ps_tr.tile([128, 512], F32, tag="pt", name="psd")[:, 0:128]
            for k3 in range(K3):
                sl = gt3[:, k3, :, m * 128:(m + 1) * 128]
                nc.tensor.matmul(psd, lhsT=sl, rhs=sl,
                                 start=(k3 == 0), stop=(k3 == K3 - 1),
                                 perf_mode=DR)
            scr_d = scrp.tile([128, 128], BF16, tag="scr_ex", name="scr_d")
            nc.vector.scalar_tensor_tensor(
                out=scr_d, in0=psd, scalar=1.0, in1=eye,
                op0=AL.mult, op1=AL.mult, accum_out=ssg[:, m:m + 1],
            )
            if m % 4 == 3:
                hs = slice(m - 3, m + 1)
                nc.scalar.activation(lng[:, hs], ssg[:, hs], AF.Ln)
                nc.scalar.activation(invg[:, hs], lng[:, hs], AF.Exp,
                                     scale=-0.5)
        st.update(ssg=ssg, invg=invg)

    def emit_praw(b, st):
        # deferred into the NEXT iteration: praw would otherwise HOL-block
        # the next batch's de/e-chain on the DVE queue
        g_nat, e_nat = nat_tiles[b]
        praw = small.tile([128, MC], F32, tag="praw", name="praw")
        for m in range(MC):
            gs = g_nat[:, m * H:(m + 1) * H]
            es = e_nat[:, m * H:(m + 1) * H]
            scr_p = scrp.tile([128, H], BF16, tag="scr_pr", name="scr_p")
            nc.vector.scalar_tensor_tensor(
                out=scr_p, in0=gs, scalar=1.0, in1=es,
                op0=AL.mult, op1=AL.mult, accum_out=praw[:, m:m + 1],
            )
        pos = small.tile([128, MC], F32, tag="pos", name="pos")
        nc.vector.tensor_mul(pos, praw, st["invg"])
        nc.vector.tensor_mul(pos, pos, st["inve_T"])
        st.update(pos=pos)

    def emit_sims(b, st):
        gt3, et, invg = st["gt3"], st["et"], st["invg"]
        et3 = et.rearrange("q (k3 ko t) -> q k3 ko t", ko=2, t=P)
        s_col = small.tile([128, MC], F32, tag="s_col", name="s_col")
        for m in range(MC):
            ps = ps_sim.tile([128, P], F32, tag="ps", name="ps")
            for k3 in range(K3):
                for half in range(2):
                    nc.tensor.matmul(
                        ps[:, half * 512:(half + 1) * 512],
                        lhsT=gt3[:, k3, :, m * 128:(m + 1) * 128],
                        rhs=et3[:, k3, :, half * 512: half * 512 + 512],
                        start=(k3 == 0), stop=(k3 == K3 - 1),
                        perf_mode=DR,
                    )
            esc = expp.tile([128, P], BF16, tag="esc", name="esc")
            nc.scalar.activation(out=esc, in_=ps, func=AF.Exp,
                                 scale=invg[:, m:m + 1],
                                 accum_out=s_col[:, m:m + 1])
        st.update(s_col=s_col)

    def emit_tail(b, st):
        # row_loss = ln(1 + s * exp(-pos)), masked by pos & batch_ok
        bc_sb, pos, posm, s_col = st["bc_sb"], st["pos"], st["posm"], st["s_col"]
        s_adj = small.tile([128, MC], F32, tag="s_adj", name="s_adj")
        nc.vector.tensor_scalar(s_adj, s_col, bc_sb[:, 1:2], None, AL.subtract)
        tn = small.tile([128, MC], F32, tag="tn", name="tn")
        nc.scalar.activation(tn, pos, AF.Exp, scale=-1.0)
        u = small.tile([128, MC], F32, tag="u", name="u")
        nc.vector.tensor_mul(u, s_adj, tn)
        v = small.tile([128, MC], F32, tag="v", name="v")
        nc.scalar.activation(v, u, AF.Ln, bias=1.0)
        meff = small.tile([128, MC], F32, tag="meff", name="meff")
        nc.vector.tensor_scalar(meff, posm, bc_sb[:, 0:1], None, AL.mult)
        scr8 = small.tile([128, MC], F32, tag="scr8", name="scr8")
        nc.vector.scalar_tensor_tensor(
            out=scr8, in0=v, scalar=1.0, in1=meff,
            op0=AL.mult, op1=AL.mult, accum_out=con_sum_parts[:, b:b + 1],
        )
        nc.vector.tensor_reduce(con_cnt_parts[:, b:b + 1], meff, AX.X, AL.add)

    prev = None
    for b in range(BPC):
        st = emit_masks(b)
        emit_sse(b, st)
        emit_echain(b, st)
        emit_etrans(b, st)
        if prev is not None:
            emit_praw(b - 1, prev)
        emit_gtrans(b, st)
        emit_ssgpe(b, st)
        if b + 2 < BPC:
            emit_dma(b + 2)
        if prev is not None:
            emit_tail(b - 1, prev)
        emit_sims(b, st)
        prev = st
    emit_praw(BPC - 1, prev)
    emit_tail(BPC - 1, prev)

    # ---------------- final partition reduction -----------------
    nc.vector.tensor_reduce(acc4[:, 2:3], con_sum_parts, AX.X, AL.add)
    nc.vector.tensor_reduce(acc4[:, 3:4], con_cnt_parts, AX.X, AL.add)
    ps_fin = ps_sm.tile([128, 8], F32, tag="sm")
    nc.tensor.matmul(ps_fin[0:1, 0:4], lhsT=ones_col, rhs=acc4,
                     start=True, stop=True)
    outsb = consts.tile([1, 4], F32)
    nc.vector.tensor_copy(outsb, ps_fin[0:1, 0:4])
    nc.sync.dma_start(out=out_d, in_=outsb)


def build_nc():
    nc = bacc.Bacc("TRN2", target_bir_lowering=False, debug=False)
    g_d = nc.dram_tensor("g", [BPC, P, H], F32, kind="ExternalInput").ap()
    e_d = nc.dram_tensor("e", [BPC, P, H], F32, kind="ExternalInput").ap()
    lg_d = nc.dram_tensor("lg", [BPC, P, 2], F32, kind="ExternalInput").ap()
    lab_d = nc.dram_tensor("lab", [BPC, P], F32, kind="ExternalInput").ap()
    eye_d = nc.dram_tensor("eye", [128, 128], F32, kind="ExternalInput").ap()
    out_d = nc.dram_tensor("out", [1, 4], F32, kind="ExternalOutput").ap()
    with tile.TileContext(nc) as tc:
        with ExitStack() as ctx:
            _emit(ctx, tc, out_d, g_d, e_d, lg_d, lab_d, eye_d)
    nc.compile()
    return nc


_NC_CACHE = {}


def _setup_pruned_act_tables():
    """Point walrus at an act-table dir containing only the one function set
    we use (exp/ln/square/copy), so it never ping-pongs ACT_TABLE_LOADs."""
    if os.environ.get("BASS_ACT_ROOT_JSON_PATH"):
        return
    try:
        import json
        import tempfile
        from neuronxcc.driver.Job import Job
        from neuronxcc.driver.jobs.support.FindActInfo import findActInfoFile
        src = findActInfoFile(Job.getPackageDir(), "gen3")
        src_dir = os.path.dirname(src)
        dst = os.path.join(tempfile.gettempdir(), "act_pruned_nle")
        os.makedirs(dst, exist_ok=True)
        for f in os.listdir(src_dir):
            d = os.path.join(dst, f)
            if not os.path.exists(d):
                os.symlink(os.path.join(src_dir, f), d)
        info = json.load(open(src))
        keep = [x for x in info["act_func_sets"]
                if x["name"] == "natural_log_exp_and_others"]
        if not keep:
            return
        info["act_func_sets"] = keep
        pruned = os.path.join(dst, "act_info.json")
        if os.path.islink(pruned) or os.path.exists(pruned):
            os.remove(pruned)
        json.dump(info, open(pruned, "w"))
        os.environ["BASS_ACT_ROOT_JSON_PATH"] = pruned

        # Bacc pre-places the table loads with set ids indexing the SAME
        # json walrus sees — patch its table source to the pruned file.
        import concourse.hw_specs as hw_specs
        if not getattr(hw_specs, "_act_tables_pruned", False):
            def _pruned_tables(module_arch, _p=pruned, _mb=mybir):
                with open(_p) as af:
                    ai = json.load(af)
                return {
                    ent["name"]: {
                        _mb.ActivationFunctionType.from_pwp(a)
                        for a in ent["act"].keys()
                    }
                    for ent in ai["act_func_sets"]
                }
            hw_specs.get_activation_tables = _pruned_tables
            bacc.get_activation_tables = _pruned_tables
            hw_specs._act_tables_pruned = True
    except Exception:
        os.environ.pop("BASS_ACT_ROOT_JSON_PATH", None)  # fall back to default


def _get_nc():
    if "nc" not in _NC_CACHE:
        _setup_pruned_act_tables()
        _NC_CACHE["nc"] = build_nc()
    return _NC_CACHE["nc"]


def make_in_maps(logits, labels, greek_embeds, english_embeds):
    logits = np.ascontiguousarray(np.asarray(logits), dtype=np.float32)
    labf = np.ascontiguousarray(np.asarray(labels)).astype(np.float32)
    g = np.ascontiguousarray(np.asarray(greek_embeds), dtype=np.float32)
    e = np.ascontiguousarray(np.asarray(english_embeds), dtype=np.float32)
    eye = np.eye(128, dtype=np.float32)
    in_maps = []
    for c in range(NCORES):
        sl = slice(c * BPC, (c + 1) * BPC)
        in_maps.append({
            "g": np.ascontiguousarray(g[sl]),
            "e": np.ascontiguousarray(e[sl]),
            "lg": np.ascontiguousarray(logits[sl]),
            "lab": np.ascontiguousarray(labf[sl]),
            "eye": eye,
        })
    return in_maps


def combine_outputs(results):
    parts = np.stack([np.asarray(r["out"]).reshape(4) for r in results]).astype(np.float64)
    cls_sum, cls_cnt, con_sum, con_cnt = parts.sum(axis=0)
    cls = cls_sum / max(cls_cnt, 1.0)
    con = 0.0 if con_cnt == 0 else con_sum / max(con_cnt, 1.0)
    return np.float32(1.0 * cls + 0.5 * con)


def kernel(logits, labels, greek_embeds, english_embeds):
    from concourse import bass_utils

    nc = _get_nc()
    in_maps = make_in_maps(logits, labels, greek_embeds, english_embeds)
    res = bass_utils.run_bass_kernel_spmd(nc, in_maps, core_ids=list(range(NCORES)))
    return combine_outputs(res.results)


# revision 33
# speedup vs baseline: 1.1186x; 1.0432x over previous
"""Trainium2 Bass kernel for nn_CombinedLoss (CE + contrastive loss).

Data-parallel over the batch dim: 4 batches per core on 8 NeuronCores.
Each core returns partial (cls_sum, cls_cnt, con_sum, con_cnt); the host
reduces across cores and performs the final divisions.

v2: fp8e4+DoubleRow sim matmuls, raw-g transpose (g's norm folded into
the exp's per-row scale), chunked HWDGE DMA, elementwise work spread
across DVE/ACT/Pool.
"""

import os
import sys

for _p in ("/opt/trn_rl_repo", "/root/.axon_site/_ro/trn_rl_repo"):
    if os.path.isdir(_p) and _p not in sys.path:
        sys.path.insert(0, _p)

import math
from contextlib import ExitStack

import numpy as np

import concourse.bass as bass
import concourse.bacc as bacc
import concourse.tile as tile
from concourse import mybir

B, P, H = 32, 1024, 768
NCORES = 8
BPC = B // NCORES          # batches per core
MC = P // 128              # 128-token chunks per batch
KC = H // 128              # 128-dim contraction chunks
K3 = KC // 2               # 256-dim DoubleRow contraction chunks
TEMP = 0.07
F32 = mybir.dt.float32
BF16 = mybir.dt.bfloat16
F8 = mybir.dt.float8e4
DR = mybir.MatmulPerfMode.DoubleRow


def _emit(ctx, tc, out_d, g_d, e_d, lg_d, lab_d, eye_d):
    nc = tc.nc
    AL = mybir.AluOpType
    AF = mybir.ActivationFunctionType
    AX = mybir.AxisListType

    consts = ctx.enter_context(tc.tile_pool(name="consts", bufs=1))
    nat = ctx.enter_context(tc.tile_pool(name="nat", bufs=4))
    trans = ctx.enter_context(tc.tile_pool(name="trans", bufs=2))
    diagp = ctx.enter_context(tc.tile_pool(name="diagp", bufs=2))
    small = ctx.enter_context(tc.tile_pool(name="small", bufs=2))
    scrp = ctx.enter_context(tc.tile_pool(name="scrp", bufs=4))
    expp = ctx.enter_context(tc.tile_pool(name="expp", bufs=2))
    ps_sim = ctx.enter_context(tc.tile_pool(name="ps_sim", bufs=2, space="PSUM"))
    ps_tr = ctx.enter_context(tc.tile_pool(name="ps_tr", bufs=3, space="PSUM"))
    ps_sm = ctx.enter_context(tc.tile_pool(name="ps_sm", bufs=1, space="PSUM"))

    # small latency-sensitive DMAs first, on the sync (HWDGE) ring
    eye = consts.tile([128, 128], F32)
    nc.sync.dma_start(out=eye, in_=eye_d)
    eye_bf = consts.tile([128, 128], BF16)
    nc.gpsimd.dma_start(out=eye_bf, in_=eye_d)    # cast f32 -> bf16 (SWDGE)
    lgt = consts.tile([128, 2 * P * BPC // 128], F32)          # [128, 64]
    nc.sync.dma_start(
        out=lgt,
        in_=lg_d.rearrange("b p y -> (b p y)").rearrange("(q f) -> q f", q=128),
    )
    labfl = consts.tile([128, P * BPC // 128], F32)            # [128, 32]
    nc.sync.dma_start(
        out=labfl,
        in_=lab_d.rearrange("b p -> (b p)").rearrange("(q f) -> q f", q=128),
    )
    # all batches' labels in [m, (b q)] layout, one upfront DMA
    lab8x4 = consts.tile([8, BPC * 128], F32)
    nc.sync.dma_start(
        out=lab8x4.rearrange("m (b q) -> m b q", q=128),
        in_=lab_d.rearrange("b (m q) -> m b q", q=128),
    )
    ones_col = consts.tile([128, 1], F32)
    nc.vector.memset(ones_col, 1.0)
    ones_row = consts.tile([1, 128], F32)
    nc.vector.memset(ones_row, 1.0)

    c_lnT = consts.tile([128, 1], F32)                 # ln(1/TEMP) bias for ACT
    nc.vector.memset(c_lnT, float(math.log(1.0 / TEMP)))

    # column accumulators; final partition-sum happens once at the end
    acc4 = consts.tile([128, 4], F32)          # cls_sum | cls_cnt | con_sum | con_cnt
    con_sum_parts = consts.tile([128, BPC], F32)
    con_cnt_parts = consts.tile([128, BPC], F32)

    # ---------------- nat DMA, chunked on the gpsimd (SWDGE) ring -----------------
    nat_tiles = {}

    def emit_dma(b):
        # e entirely first: its arrival gates the critical chain
        # (sse -> inve -> de -> e-trans); g only feeds later stages
        g_nat = nat.tile([128, MC * H], BF16, tag="g_nat", name="g_nat")
        e_nat = nat.tile([128, MC * H], BF16, tag="e_nat", name="e_nat")
        for nt, dd, nsplit in ((e_nat, e_d, 4), (g_nat, g_d, 2)):
            mm = MC // nsplit
            for hh in range(nsplit):
                nc.gpsimd.dma_start(
                    out=nt[:, hh * mm * H:(hh + 1) * mm * H]
                        .rearrange("q (m h) -> q m h", m=mm),
                    in_=dd[b][hh * mm * 128:(hh + 1) * mm * 128]
                        .rearrange("(m q) h -> q m h", q=128))
        nat_tiles[b] = (g_nat, e_nat)

    emit_dma(0)
    emit_dma(1)

    # ---------------- classification CE (tiny, fills the ramp) -----------------
    lg3 = lgt.rearrange("q (t y) -> q t y", y=2)
    x0 = lg3[:, :, 0:1].rearrange("q t y -> q (t y)")          # [128, 32] strided
    x1 = lg3[:, :, 1:2].rearrange("q t y -> q (t y)")

    nctok = P * BPC // 128                                     # 32
    e0 = consts.tile([128, nctok], F32)
    nc.scalar.activation(e0, x0, AF.Exp)
    e1 = consts.tile([128, nctok], F32)
    nc.scalar.activation(e1, x1, AF.Exp)
    se = consts.tile([128, nctok], F32)
    nc.vector.tensor_add(se, e0, e1)
    lae = consts.tile([128, nctok], F32)
    nc.scalar.activation(lae, se, AF.Ln)                   # logaddexp(x0, x1)
    validm = consts.tile([128, nctok], F32)
    nc.vector.tensor_scalar(validm, labfl, 0.0, None, AL.is_ge)
    tv = consts.tile([128, nctok], F32)
    nc.vector.tensor_mul(tv, labfl, validm)                # target as {0,1}
    d10 = consts.tile([128, nctok], F32)
    nc.vector.tensor_sub(d10, x1, x0)
    td = consts.tile([128, nctok], F32)
    nc.vector.tensor_mul(td, tv, d10)
    xt = consts.tile([128, nctok], F32)
    nc.vector.tensor_add(xt, x0, td)                       # x_target
    ce = consts.tile([128, nctok], F32)
    nc.vector.tensor_sub(ce, lae, xt)
    clsscr = consts.tile([128, nctok], F32)
    nc.vector.scalar_tensor_tensor(
        out=clsscr, in0=ce, scalar=1.0, in1=validm,
        op0=AL.mult, op1=AL.mult, accum_out=acc4[:, 0:1],
    )
    nc.vector.tensor_reduce(acc4[:, 1:2], validm, AX.X, AL.add)

    # ---------------- contrastive loss, software-pipelined -----------------
    # gt: raw g transposed (norm applied later via the exp's per-row scale)
    # et: e transposed, scaled per-token by negmask * inv_norm_e / TEMP
    # Both fp8e4 with DoubleRow layout [128, (kk=k3*2+ko)*1024 + tok].
    # ssg comes from diag(gt.T @ gt) on the PE (fp8-consistent norms).

    def emit_masks(b):
        st = {}
        # labels in column form [128, 8]: token 128*m + p at [p, m]
        ps_lab = ps_sm.tile([128, 8], F32, tag="sm", name="ps_lab")
        nc.tensor.transpose(ps_lab, lab8x4[:, b * 128:(b + 1) * 128],
                            eye[0:8, 0:8])
        lab_col = small.tile([128, MC], F32, tag="lab_col", name="lab_col")
        nc.vector.tensor_copy(lab_col, ps_lab)
        posm = small.tile([128, MC], F32, tag="posm", name="posm")
        nc.vector.tensor_scalar(posm, lab_col, 1.0, None, AL.is_equal)
        negm = small.tile([128, MC], F32, tag="negm", name="negm")
        nc.vector.tensor_scalar(negm, lab_col, 0.0, None, AL.is_equal)
        st.update(posm=posm, negm=negm)
        return st

    def emit_sse(b, st):
        e_nat = nat_tiles[b][1]
        sse = small.tile([128, MC], F32, tag="sse", name="sse")
        for m in range(MC):
            es = e_nat[:, m * H:(m + 1) * H]
            if m % 2 == 0:
                scr_e = scrp.tile([128, H], BF16, tag="scr_act", name="scr_e")
                nc.scalar.activation(out=scr_e, in_=es, func=AF.Square,
                                     accum_out=sse[:, m:m + 1])
            else:
                scr_e = scrp.tile([128, H], BF16, tag="scr_dve", name="scr_e")
                nc.vector.scalar_tensor_tensor(
                    out=scr_e, in0=es, scalar=1.0, in1=es,
                    op0=AL.mult, op1=AL.mult, accum_out=sse[:, m:m + 1],
                )
        st.update(sse=sse)

    def emit_echain(b, st):
        sse, negm, posm = st["sse"], st["negm"], st["posm"]
        # inve_T = exp(-0.5 ln sse + ln(1/T)); inve_eff = inve_T * negm
        lne = small.tile([128, MC], F32, tag="lne", name="lne")
        inve_T = small.tile([128, MC], F32, tag="inve_T", name="inve_T")
        inve_eff = small.tile([128, MC], F32, tag="inve_eff", name="inve_eff")
        nc.scalar.activation(lne, sse, AF.Ln)
        nc.scalar.activation(inve_T, lne, AF.Exp, scale=-0.5, bias=c_lnT)
        nc.vector.tensor_mul(inve_eff, inve_T, negm)

        # per-token diagonal scale matrix for e's fused scaled transpose
        de = diagp.tile([128, MC * 128], BF16, tag="de", name="de")
        for m in range(MC):
            nc.vector.tensor_scalar(de[:, m * 128:(m + 1) * 128], eye_bf,
                                    inve_eff[:, m:m + 1], None, AL.mult)

        # batch_ok and the "zeroed columns contribute exp(0)=1" correction
        cnt2 = small.tile([128, 2], F32, tag="cnt2", name="cnt2")
        nc.vector.tensor_reduce(cnt2[:, 0:1], negm, AX.X, AL.add)
        nc.vector.tensor_reduce(cnt2[:, 1:2], posm, AX.X, AL.add)
        ps_cnt = ps_sm.tile([128, 8], F32, tag="sm", name="ps_cnt")
        nc.tensor.matmul(ps_cnt[0:1, 0:2], lhsT=ones_col, rhs=cnt2,
                         start=True, stop=True)
        cnt_sb = small.tile([1, 2], F32, tag="cnt_sb", name="cnt_sb")
        nc.vector.tensor_copy(cnt_sb, ps_cnt[0:1, 0:2])
        mn = small.tile([1, 2], F32, tag="mn", name="mn")
        nc.vector.tensor_scalar(mn, cnt_sb, 1.0, None, AL.min)
        okn = small.tile([1, 2], F32, tag="okn", name="okn")  # [ok, P - n_neg]
        nc.vector.tensor_mul(okn[:, 0:1], mn[:, 0:1], mn[:, 1:2])
        nc.vector.tensor_scalar(okn[:, 1:2], cnt_sb[:, 0:1], -1.0, float(P),
                                AL.mult, AL.add)
        ps_bc = ps_sm.tile([128, 8], F32, tag="sm", name="ps_bc")
        nc.tensor.matmul(ps_bc[:, 0:2], lhsT=ones_row, rhs=okn,
                         start=True, stop=True)
        bc_sb = small.tile([128, 2], F32, tag="bc_sb", name="bc_sb")
        nc.vector.tensor_copy(bc_sb, ps_bc[:, 0:2])
        st.update(inve_T=inve_T, bc_sb=bc_sb, de=de)

    def emit_etrans(b, st):
        e_nat = nat_tiles[b][1]
        de = st["de"]
        et = trans.tile([128, KC * P], F8, tag="et", name="et")
        cp = 0
        for half in range(2):       # h0 chunks first: matches DMA arrival
            for kk in range(KC):
                pt = ps_tr.tile([128, 512], F32, tag="pt", name="pt")
                for mi in range(4):
                    m = half * 4 + mi
                    nc.tensor.matmul(
                        pt[:, mi * 128:(mi + 1) * 128],
                        lhsT=e_nat[:, m * H + kk * 128: m * H + (kk + 1) * 128],
                        rhs=de[:, m * 128:(m + 1) * 128],
                        start=True, stop=True,
                    )
                dst = et[:, kk * P + half * 512: kk * P + half * 512 + 512]
                if cp % 2 == 0:
                    nc.scalar.copy(out=dst, in_=pt)
                else:
                    nc.vector.tensor_copy(dst, pt)
                cp += 1
        st.update(et=et)

    def emit_gtrans(b, st):
        # unscaled transpose of raw g into fp8 DoubleRow layout
        g_nat = nat_tiles[b][0]
        gt = trans.tile([128, KC * P], F8, tag="gt", name="gt")
        cp = 0
        for half in range(2):       # h0 chunks first: matches DMA arrival
            for kk in range(KC):
                pt = ps_tr.tile([128, 512], F32, tag="pt", name="pt")
                for mi in range(4):
                    m = half * 4 + mi
                    nc.tensor.matmul(
                        pt[:, mi * 128:(mi + 1) * 128],
                        lhsT=g_nat[:, m * H + kk * 128: m * H + (kk + 1) * 128],
                        rhs=eye_bf,
                        start=True, stop=True,
                    )
                dst = gt[:, kk * P + half * 512: kk * P + half * 512 + 512]
                if cp % 2 == 0:
                    nc.scalar.copy(out=dst, in_=pt)
                else:
                    nc.vector.tensor_copy(dst, pt)
                cp += 1
        st.update(gt3=gt.rearrange("q (k3 ko t) -> q k3 ko t", ko=2, t=P))

    def emit_ssgpe(b, st):
        # ssg[t] = sum_h gt[h,t]^2 = diag(gt.T @ gt), DR matmuls + eye-masked
        # extraction; invg computed per half so exps can start early
        gt3 = st["gt3"]
        ssg = small.tile([128, MC], F32, tag="ssg", name="ssg")
        invg = small.tile([128, MC], F32, tag="invg", name="invg")
        lng = small.tile([128, MC], F32, tag="lng", name="lng")
        for m in range(MC):
            psd = ps_tr.tile([128, 512], F32, tag="pt", name="psd")[:, 0:128]
            for k3 in range(K3):
                sl = gt3[:, k3, :, m * 128:(m + 1) * 128]
                nc.tensor.matmul(psd, lhsT=sl, rhs=sl,
                                 start=(k3 == 0), stop=(k3 == K3 - 1),
                                 perf_mode=DR)
            scr_d = scrp.tile([128, 128], BF16, tag="scr_ex", name="scr_d")
            nc.vector.scalar_tensor_tensor(
                out=scr_d, in0=psd, scalar=1.0, in1=eye,
                op0=AL.mult, op1=AL.mult, accum_out=ssg[:, m:m + 1],
            )
            if m % 4 == 3:
                hs = slice(m - 3, m + 1)
                nc.scalar.activation(lng[:, hs], ssg[:, hs], AF.Ln)
                nc.scalar.activation(invg[:, hs], lng[:, hs], AF.Exp,
                                     scale=-0.5)
        st.update(ssg=ssg, invg=invg)

    def emit_praw(b, st):
        # deferred into the NEXT iteration: praw would otherwise HOL-block
        # the next batch's de/e-chain on the DVE queue
        g_nat, e_nat = nat_tiles[b]
        praw = small.tile([128, MC], F32, tag="praw", name="praw")
        for m in range(MC):
            gs = g_nat[:, m * H:(m + 1) * H]
            es = e_nat[:, m * H:(m + 1) * H]
            scr_p = scrp.tile([128, H], BF16, tag="scr_pr", name="scr_p")
            nc.vector.scalar_tensor_tensor(
                out=scr_p, in0=gs, scalar=1.0, in1=es,
                op0=AL.mult, op1=AL.mult, accum_out=praw[:, m:m + 1],
            )
        pos = small.tile([128, MC], F32, tag="pos", name="pos")
        nc.vector.tensor_mul(pos, praw, st["invg"])
        nc.vector.tensor_mul(pos, pos, st["inve_T"])
        st.update(pos=pos)

    def emit_sims(b, st):
        gt3, et, invg = st["gt3"], st["et"], st["invg"]
        et3 = et.rearrange("q (k3 ko t) -> q k3 ko t", ko=2, t=P)
        s_col = small.tile([128, MC], F32, tag="s_col", name="s_col")
        for m in range(MC):
            ps = ps_sim.tile([128, P], F32, tag="ps", name="ps")
            for k3 in range(K3):
                for half in range(2):
                    nc.tensor.matmul(
                        ps[:, half * 512:(half + 1) * 512],
                        lhsT=gt3[:, k3, :, m * 128:(m + 1) * 128],
                        rhs=et3[:, k3, :, half * 512: half * 512 + 512],
                        start=(k3 == 0), stop=(k3 == K3 - 1),
                        perf_mode=DR,
                    )
            esc = expp.tile([128, P], BF16, tag="esc", name="esc")
            nc.scalar.activation(out=esc, in_=ps, func=AF.Exp,
                                 scale=invg[:, m:m + 1],
                                 accum_out=s_col[:, m:m + 1])
        st.update(s_col=s_col)

    def emit_tail(b, st):
        # row_loss = ln(1 + s * exp(-pos)), masked by pos & batch_ok
        bc_sb, pos, posm, s_col = st["bc_sb"], st["pos"], st["posm"], st["s_col"]
        s_adj = small.tile([128, MC], F32, tag="s_adj", name="s_adj")
        nc.vector.tensor_scalar(s_adj, s_col, bc_sb[:, 1:2], None, AL.subtract)
        tn = small.tile([128, MC], F32, tag="tn", name="tn")
        nc.scalar.activation(tn, pos, AF.Exp, scale=-1.0)
        u = small.tile([128, MC], F32, tag="u", name="u")
        nc.vector.tensor_mul(u, s_adj, tn)
        v = small.tile([128, MC], F32, tag="v", name="v")
        nc.scalar.activation(v, u, AF.Ln, bias=1.0)
        meff = small.tile([128, MC], F32, tag="meff", name="meff")
        nc.vector.tensor_scalar(meff, posm, bc_sb[:, 0:1], None, AL.mult)
        scr8 = small.tile([128, MC], F32, tag="scr8", name="scr8")
        nc.vector.scalar_tensor_tensor(
            out=scr8, in0=v, scalar=1.0, in1=meff,
            op0=AL.mult, op1=AL.mult, accum_out=con_sum_parts[:, b:b + 1],
        )
        nc.vector.tensor_reduce(con_cnt_parts[:, b:b + 1], meff, AX.X, AL.add)

    prev = None
    for b in range(BPC):
        if b + 2 < BPC:
            emit_dma(b + 2)   # bufs=4: no WAR wait, ring never starves
        st = emit_masks(b)
        emit_sse(b, st)
        emit_echain(b, st)
        emit_etrans(b, st)
        if prev is not None:
            emit_praw(b - 1, prev)
        emit_gtrans(b, st)
        emit_ssgpe(b, st)
        if prev is not None:
            emit_tail(b - 1, prev)
        emit_sims(b, st)
        prev = st
    emit_praw(BPC - 1, prev)
    emit_tail(BPC - 1, prev)

    # ---------------- final partition reduction -----------------
    nc.vector.tensor_reduce(acc4[:, 2:3], con_sum_parts, AX.X, AL.add)
    nc.vector.tensor_reduce(acc4[:, 3:4], con_cnt_parts, AX.X, AL.add)
    ps_fin = ps_sm.tile([128, 8], F32, tag="sm")
    nc.tensor.matmul(ps_fin[0:1, 0:4], lhsT=ones_col, rhs=acc4,
                     start=True, stop=True)
    outsb = consts.tile([1, 4], F32)
    nc.vector.tensor_copy(outsb, ps_fin[0:1, 0:4])
    nc.sync.dma_start(out=out_d, in_=outsb)


def build_nc():
    nc = bacc.Bacc("TRN2", target_bir_lowering=False, debug=False)
    g_d = nc.dram_tensor("g", [BPC, P, H], F32, kind="ExternalInput").ap()
    e_d = nc.dram_tensor("e", [BPC, P, H], F32, kind="ExternalInput").ap()
    lg_d = nc.dram_tensor("lg", [BPC, P, 2], F32, kind="ExternalInput").ap()
    lab_d = nc.dram_tensor("lab", [BPC, P], F32, kind="ExternalInput").ap()
    eye_d = nc.dram_tensor("eye", [128, 128], F32, kind="ExternalInput").ap()
    out_d = nc.dram_tensor("out", [1, 4], F32, kind="ExternalOutput").ap()
    with tile.TileContext(nc) as tc:
        with ExitStack() as ctx:
            _emit(ctx, tc, out_d, g_d, e_d, lg_d, lab_d, eye_d)
    nc.compile()
    return nc


_NC_CACHE = {}


def _setup_pruned_act_tables():
    """Point walrus at an act-table dir containing only the one function set
    we use (exp/ln/square/copy), so it never ping-pongs ACT_TABLE_LOADs."""
    if os.environ.get("BASS_ACT_ROOT_JSON_PATH"):
        return
    try:
        import json
        import tempfile
        from neuronxcc.driver.Job import Job
        from neuronxcc.driver.jobs.support.FindActInfo import findActInfoFile
        src = findActInfoFile(Job.getPackageDir(), "gen3")
        src_dir = os.path.dirname(src)
        dst = os.path.join(tempfile.gettempdir(), "act_pruned_nle")
        os.makedirs(dst, exist_ok=True)
        for f in os.listdir(src_dir):
            d = os.path.join(dst, f)
            if not os.path.exists(d):
                os.symlink(os.path.join(src_dir, f), d)
        info = json.load(open(src))
        keep = [x for x in info["act_func_sets"]
                if x["name"] == "natural_log_exp_and_others"]
        if not keep:
            return
        info["act_func_sets"] = keep
        pruned = os.path.join(dst, "act_info.json")
        if os.path.islink(pruned) or os.path.exists(pruned):
            os.remove(pruned)
        json.dump(info, open(pruned, "w"))
        os.environ["BASS_ACT_ROOT_JSON_PATH"] = pruned

        # Bacc pre-places the table loads with set ids indexing the SAME
        # json walrus sees — patch its table source to the pruned file.
        import concourse.hw_specs as hw_specs
        if not getattr(hw_specs, "_act_tables_pruned", False):
            def _pruned_tables(module_arch, _p=pruned, _mb=mybir):
                with open(_p) as af:
                    ai = json.load(af)
                return {
                    ent["name"]: {
                        _mb.ActivationFunctionType.from_pwp(a)
                        for a in ent["act"].keys()
                    }
                    for ent in ai["act_func_sets"]
                }
            hw_specs.get_activation_tables = _pruned_tables
            bacc.get_activation_tables = _pruned_tables
            hw_specs._act_tables_pruned = True
    except Exception:
        os.environ.pop("BASS_ACT_ROOT_JSON_PATH", None)  # fall back to default


def _get_nc():
    if "nc" not in _NC_CACHE:
        _setup_pruned_act_tables()
        _NC_CACHE["nc"] = build_nc()
    return _NC_CACHE["nc"]


def make_in_maps(logits, labels, greek_embeds, english_embeds):
    logits = np.ascontiguousarray(np.asarray(logits), dtype=np.float32)
    labf = np.ascontiguousarray(np.asarray(labels)).astype(np.float32)
    g = np.ascontiguousarray(np.asarray(greek_embeds), dtype=np.float32)
    e = np.ascontiguousarray(np.asarray(english_embeds), dtype=np.float32)
    eye = np.eye(128, dtype=np.float32)
    in_maps = []
    for c in range(NCORES):
        sl = slice(c * BPC, (c + 1) * BPC)
        in_maps.append({
            "g": np.ascontiguousarray(g[sl]),
            "e": np.ascontiguousarray(e[sl]),
            "lg": np.ascontiguousarray(logits[sl]),
            "lab": np.ascontiguousarray(labf[sl]),
            "eye": eye,
        })
    return in_maps


def combine_outputs(results):
    parts = np.stack([np.asarray(r["out"]).reshape(4) for r in results]).astype(np.float64)
    cls_sum, cls_cnt, con_sum, con_cnt = parts.sum(axis=0)
    cls = cls_sum / max(cls_cnt, 1.0)
    con = 0.0 if con_cnt == 0 else con_sum / max(con_cnt, 1.0)
    return np.float32(1.0 * cls + 0.5 * con)


def kernel(logits, labels, greek_embeds, english_embeds):
    from concourse import bass_utils

    nc = _get_nc()
    in_maps = make_in_maps(logits, labels, greek_embeds, english_embeds)
    res = bass_utils.run_bass_kernel_spmd(nc, in_maps, core_ids=list(range(NCORES)))
    return combine_outputs(res.results)
